# revision 9
# baseline (speedup 1.0000x reference)
"""Multi-head attention (B=2, S=2048, H=1024, NH=16, HD=64) on 8 trn2 cores.

Sharding: tensor-parallel over heads. Core c owns heads {2c, 2c+1}, i.e.
feature columns [128c, 128c+128) of q/k/v. Wq/Wk/Wv are column-sharded,
Wo row-sharded; each core computes a full-shape partial output and the
host sums the 8 partials (the row-parallel reduce) during unshard.

On-chip layout is feature-major ("transposed"): the host passes
hsT = hidden_states.T so both matmul operands of every projection have
the contraction dim on partitions and no on-chip transposes of big
tensors are needed. Attention works on scoresT[tk, tq]; softmax's
normalizer comes from a ones-column augmented V matmul (exp is safe
without max-subtraction because scores are O(6) here).

All matmuls run in float32r (fast fp32 mode, ~1.4e-4 rel accuracy);
accumulation is full fp32 in PSUM.
"""

import numpy as np

B, S, H, NH, HD = 2, 2048, 1024, 16, 64
NCORES = 8
JC = 128  # head-columns per core (2 heads x 64)
T = B * S  # 4096 tokens
TQB = 512  # tq block
NKT = S // 128  # 16 tk blocks per batch
WAVE = 1024  # qkv projection token-chunk per wave
BASE = 10000.0

_nc_cache = [None]

_LDW_OPT = True


def _patch_ldw_opt():
    from concourse import bass_utils as _bu

    if getattr(_bu, "_ldw_patched", False):
        return
    _orig = _bu.run_command

    def _patched(argv, **kw):
        argv = [
            a.replace("--enable-ldw-opt=false", "--enable-ldw-opt=true")
            if _LDW_OPT and isinstance(a, str)
            else a
            for a in argv
        ]
        return _orig(argv, **kw)

    _bu.run_command = _patched
    _bu._ldw_patched = True


def _build():
    _patch_ldw_opt()
    import concourse.tile as tile
    from concourse import bacc, mybir
    from concourse.masks import make_identity

    F32 = mybir.dt.float32
    F32R = mybir.dt.float32r
    BF16 = mybir.dt.bfloat16
    EXP = mybir.ActivationFunctionType.Exp

    nc = bacc.Bacc("TRN2", target_bir_lowering=False, debug=False)

    hsT = nc.dram_tensor("hsT", [H, T], F32R, kind="ExternalInput").ap()
    wqT = nc.dram_tensor("wqT", [H, JC], F32R, kind="ExternalInput").ap()
    wkT = nc.dram_tensor("wkT", [H, JC], F32R, kind="ExternalInput").ap()
    wvT = nc.dram_tensor("wvT", [H, JC], F32R, kind="ExternalInput").ap()
    woJI = nc.dram_tensor("woJI", [JC, H], F32R, kind="ExternalInput").ap()
    cosT = nc.dram_tensor("cosT", [128, S], F32, kind="ExternalInput").ap()
    sinTs = nc.dram_tensor("sinTs", [128, S], F32, kind="ExternalInput").ap()
    out = nc.dram_tensor("out", [T, H], F32, kind="ExternalOutput").ap()

    with tile.TileContext(nc) as tc:
        with (
            tc.tile_pool(name="wts", bufs=1) as wts,
            tc.tile_pool(name="tabs", bufs=1) as tabs,
            tc.tile_pool(name="hst", bufs=9) as hst,
            tc.tile_pool(name="qkv", bufs=1) as qkvp,
            tc.tile_pool(name="ps", bufs=3, space="PSUM") as ps,
            tc.tile_pool(name="cxp", bufs=1, space="PSUM") as cxp,
            tc.tile_pool(name="rope", bufs=2) as ropep,
            tc.tile_pool(name="vaug", bufs=1) as vaugp,
            tc.tile_pool(name="expt", bufs=3) as exptp,
            tc.tile_pool(name="ctx", bufs=1) as ctxp,
            tc.tile_pool(name="nrm", bufs=3) as nrmp,
            tc.tile_pool(name="outs", bufs=3) as outsp,
        ):
            # ---- persistent weights / tables ----
            wq_sb = wts.tile([128, 8, JC], F32R, tag="wq")
            nc.sync.dma_start(
                out=wq_sb[:], in_=wqT[:, :].rearrange("(k p) j -> p k j", p=128)
            )
            wk_sb = wts.tile([128, 8, JC], F32R, tag="wk")
            nc.sync.dma_start(
                out=wk_sb[:], in_=wkT[:, :].rearrange("(k p) j -> p k j", p=128)
            )
            wv_sb = wts.tile([128, 8, JC], F32R, tag="wv")
            nc.sync.dma_start(
                out=wv_sb[:], in_=wvT[:, :].rearrange("(k p) j -> p k j", p=128)
            )
            wJ = wts.tile([128, H], F32R, tag="wj")
            nc.sync.dma_start(out=wJ[:], in_=woJI[:, :])
            cos_sb = tabs.tile([128, S], F32, tag="cos")
            nc.sync.dma_start(out=cos_sb[:], in_=cosT[:, :])
            sin_sb = tabs.tile([128, S], F32, tag="sin")
            nc.sync.dma_start(out=sin_sb[:], in_=sinTs[:, :])
            ident = tabs.tile([128, 128], F32, tag="ident")
            make_identity(nc, ident[:])
            onesc = tabs.tile([128, NKT], F32, tag="ones")
            nc.vector.memset(onesc[:], 1.0)
            ones64 = tabs.tile([1, 64], F32R, tag="ones64")
            nc.vector.tensor_copy(ones64[:], onesc[0:1, 0:1].to_broadcast([1, 64]))

            for b in range(B):
                # ======== QKV projections (+RoPE), feature-major ========
                qT = qkvp.tile([128, S], F32R, tag="qT")
                kT = qkvp.tile([128, S], F32R, tag="kT")
                vT = qkvp.tile([128, S], F32, tag="vT")

                for wv_i in range(S // WAVE):  # 2 waves of 1024 tokens
                    t0 = b * S + wv_i * WAVE
                    sl = slice(wv_i * WAVE, (wv_i + 1) * WAVE)
                    chunks = []
                    for k in range(8):
                        c = hst.tile([128, WAVE], F32R, tag="hst")
                        nc.sync.dma_start(
                            out=c[:], in_=hsT[128 * k : 128 * (k + 1), t0 : t0 + WAVE]
                        )
                        chunks.append(c)
                    pq = ps.tile([128, WAVE], F32, tag="ps")
                    pk = ps.tile([128, WAVE], F32, tag="ps")
                    pv = ps.tile([128, WAVE], F32, tag="ps")
                    for k in range(8):
                        st, sp = k == 0, k == 7
                        for w_sb, p in ((wq_sb, pq), (wk_sb, pk), (wv_sb, pv)):
                            for hf in range(2):
                                fsl = slice(hf * 512, (hf + 1) * 512)
                                nc.tensor.matmul(
                                    p[:, fsl],
                                    w_sb[:, k, :],
                                    chunks[k][:, fsl],
                                    start=st,
                                    stop=sp,
                                )
                    # v: plain copy to SBUF (f32)
                    nc.vector.tensor_copy(vT[:, sl], pv[:])
                    # rope(q), rope(k)
                    for psrc, dstT in ((pq, qT), (pk, kT)):
                        raw = ropep.tile([128, WAVE], F32, tag="raw")
                        nc.vector.tensor_copy(raw[:], psrc[:])
                        rot = ropep.tile([128, WAVE], F32, tag="rot")
                        for h0 in (0, 64):
                            nc.sync.dma_start(
                                out=rot[h0 : h0 + 32, :], in_=raw[h0 + 32 : h0 + 64, :]
                            )
                            nc.sync.dma_start(
                                out=rot[h0 + 32 : h0 + 64, :], in_=raw[h0 : h0 + 32, :]
                            )
                        t1 = ropep.tile([128, WAVE], F32, tag="t1")
                        nc.vector.tensor_mul(t1[:], raw[:], cos_sb[:, sl])
                        t2 = ropep.tile([128, WAVE], F32, tag="t2")
                        nc.vector.tensor_mul(t2[:], rot[:], sin_sb[:, sl])
                        nc.vector.tensor_add(dstT[:, sl], t1[:], t2[:])

                # ======== v transpose -> per-head augmented V ========
                vA = vaugp.tile([128, NKT, 65], F32R, tag="vA")
                vB = vaugp.tile([128, NKT, 65], F32R, tag="vB")
                nc.vector.tensor_copy(vA[:, :, 64], onesc[:])
                nc.vector.tensor_copy(vB[:, :, 64], onesc[:])
                for tkb in range(NKT):
                    pt = ps.tile([128, WAVE], F32, tag="ps")
                    nc.tensor.transpose(
                        pt[:, 0:128], vT[:, 128 * tkb : 128 * (tkb + 1)], ident[:]
                    )
                    nc.vector.tensor_copy(vA[:, tkb, 0:64], pt[:, 0:64])
                    nc.vector.tensor_copy(vB[:, tkb, 0:64], pt[:, 64:128])

                # ======== attention: scoresT -> exp -> ctxT ========
                ctxS = ctxp.tile([128, S], F32R, tag="cts")
                ctxB = ctxp.tile([64, S], F32R, tag="ctb")
                ctxA = ctxS
                for tqb in range(S // TQB):
                    qsl = slice(tqb * TQB, (tqb + 1) * TQB)
                    for hh, (h0, vaug, ctxT) in enumerate(
                        ((0, vA, ctxA), (64, vB, ctxB))
                    ):
                        cx = cxp.tile([65, TQB], F32, tag="cx" + "ab"[hh])
                        for p in range(NKT // 2):
                            sc = ps.tile([128, 2 * TQB], F32, tag="ps")
                            for t in range(2):
                                tkb = 2 * p + t
                                nc.tensor.matmul(
                                    sc[:, t * TQB : (t + 1) * TQB],
                                    kT[h0 : h0 + 64, 128 * tkb : 128 * (tkb + 1)],
                                    qT[h0 : h0 + 64, qsl],
                                    start=True,
                                    stop=True,
                                )
                            et = exptp.tile([128, 2 * TQB], F32R, tag="et")
                            nc.scalar.activation(et[:], sc[:], EXP, scale=0.125)
                            for t in range(2):
                                tkb = 2 * p + t
                                nc.tensor.matmul(
                                    cx[:],
                                    vaug[:, tkb, :],
                                    et[:, t * TQB : (t + 1) * TQB],
                                    start=(tkb == 0),
                                    stop=(tkb == NKT - 1),
                                )
                        # ctxT[:, qsl] = cx[0:64] / Z   (Z = row 64)
                        rz = nrmp.tile([1, TQB], F32R, tag="rz")
                        with nc.allow_low_precision(reason="f32r softmax norm"):
                            nc.vector.reciprocal(rz[:], cx[64:65, :])
                        pz = ps.tile([128, WAVE], F32, tag="ps")
                        nc.tensor.matmul(
                            pz[0:64, 0:TQB], ones64[:], rz[:], start=True, stop=True
                        )
                        zrep = nrmp.tile([64, TQB], F32, tag="zrep")
                        nc.vector.tensor_copy(zrep[:], pz[0:64, 0:TQB])
                        dst = ctxT[0:64, qsl] if ctxT is ctxS else ctxT[:, qsl]
                        nc.vector.tensor_mul(dst, cx[0:64, :], zrep[:])

                nc.sync.dma_start(out=ctxS[64:128, :], in_=ctxB[:, :])

                # ======== output projection (natural-layout out) ========
                for tq8 in range(S // 128):
                    po = ps.tile([128, WAVE], F32, tag="ps")
                    csl = slice(128 * tq8, 128 * (tq8 + 1))
                    for ich in range(2):
                        isl = slice(ich * 512, (ich + 1) * 512)
                        nc.tensor.matmul(
                            po[:, isl], ctxS[:, csl], wJ[:, isl], start=True, stop=True
                        )
                    ot = outsp.tile([128, H], F32, tag="ot")
                    if tq8 % 2 == 0:
                        nc.vector.tensor_copy(ot[:], po[:])
                    else:
                        nc.scalar.copy(ot[:], po[:])
                    nc.sync.dma_start(
                        out=out[b * S + 128 * tq8 : b * S + 128 * (tq8 + 1), :],
                        in_=ot[:],
                    )

    nc.compile()
    return nc


def _rope_tables():
    inv_freq = 1.0 / (BASE ** (np.arange(0, HD, 2, dtype=np.float64) / HD))
    t = np.arange(S, dtype=np.float64)
    freqs = np.outer(t, inv_freq)  # [S, 32]
    emb = np.concatenate([freqs, freqs], -1)  # [S, 64]
    cos = np.cos(emb).T.astype(np.float32)  # [64, S]
    sin = np.sin(emb).T.astype(np.float32)
    sin_signed = sin.copy()
    sin_signed[0:32] = -sin_signed[0:32]
    cosT = np.ascontiguousarray(np.tile(cos, (2, 1)))  # [128, S]
    sinTs = np.ascontiguousarray(np.tile(sin_signed, (2, 1)))
    return cosT, sinTs


def kernel(hidden_states, Wq, Wk, Wv, Wo):
    hidden_states = np.asarray(hidden_states, np.float32)
    Wq, Wk, Wv, Wo = (np.asarray(w, np.float32) for w in (Wq, Wk, Wv, Wo))

    if _nc_cache[0] is None:
        _nc_cache[0] = _build()
    nc = _nc_cache[0]

    hsT = np.ascontiguousarray(hidden_states.reshape(T, H).T)  # [H, T]
    cosT, sinTs = _rope_tables()
    in_maps = []
    for c in range(NCORES):
        sl = slice(JC * c, JC * (c + 1))
        in_maps.append(
            {
                "hsT": hsT,
                "wqT": np.ascontiguousarray(Wq[sl, :].T),
                "wkT": np.ascontiguousarray(Wk[sl, :].T),
                "wvT": np.ascontiguousarray(Wv[sl, :].T),
                "woJI": np.ascontiguousarray(Wo[:, sl].T),
                "cosT": cosT,
                "sinTs": sinTs,
            }
        )

    from concourse.bass_utils import run_bass_kernel_spmd

    res = run_bass_kernel_spmd(nc, in_maps, core_ids=list(range(NCORES)))
    acc = np.zeros((T, H), np.float64)
    for c in range(NCORES):
        acc += res.results[c]["out"]
    return acc.astype(np.float32).reshape(B, S, H)


# revision 10
# speedup vs baseline: 1.0421x; 1.0421x over previous
"""Multi-head attention (B=2, S=2048, H=1024, NH=16, HD=64) on 8 trn2 cores.

Sharding: tensor-parallel over heads. Core c owns heads {2c, 2c+1}, i.e.
feature columns [128c, 128c+128) of q/k/v. Wq/Wk/Wv are column-sharded,
Wo row-sharded; each core computes a full-shape partial output and the
host sums the 8 partials (the row-parallel reduce) during unshard.

On-chip layout is feature-major ("transposed"): the host passes
hsT = hidden_states.T so both matmul operands of every projection have
the contraction dim on partitions and no on-chip transposes of big
tensors are needed. Attention works on scoresT[tk, tq]; softmax's
normalizer comes from a ones-column augmented V matmul (exp is safe
without max-subtraction because scores are O(6) here).

All matmuls run in float32r (fast fp32 mode, ~1.4e-4 rel accuracy);
accumulation is full fp32 in PSUM.
"""

import numpy as np

B, S, H, NH, HD = 2, 2048, 1024, 16, 64
NCORES = 8
JC = 128  # head-columns per core (2 heads x 64)
T = B * S  # 4096 tokens
TQB = 512  # tq block
NKT = S // 128  # 16 tk blocks per batch
WAVE = 1024  # qkv projection token-chunk per wave
BASE = 10000.0

_nc_cache = [None]

_LDW_OPT = False


def _patch_ldw_opt():
    from concourse import bass_utils as _bu

    if getattr(_bu, "_ldw_patched", False):
        return
    _orig = _bu.run_command

    def _patched(argv, **kw):
        argv = [
            a.replace("--enable-ldw-opt=false", "--enable-ldw-opt=true")
            if _LDW_OPT and isinstance(a, str)
            else a
            for a in argv
        ]
        return _orig(argv, **kw)

    _bu.run_command = _patched
    _bu._ldw_patched = True


def _build():
    _patch_ldw_opt()
    import concourse.tile as tile
    from concourse import bacc, mybir
    from concourse.masks import make_identity

    F32 = mybir.dt.float32
    F32R = mybir.dt.float32r
    BF16 = mybir.dt.bfloat16
    F16 = mybir.dt.float16
    EXP = mybir.ActivationFunctionType.Exp

    nc = bacc.Bacc("TRN2", target_bir_lowering=False, debug=False)

    hsT = nc.dram_tensor("hsT", [H, T], F32R, kind="ExternalInput").ap()
    wqT = nc.dram_tensor("wqT", [H, JC], F32R, kind="ExternalInput").ap()
    wkT = nc.dram_tensor("wkT", [H, JC], F32R, kind="ExternalInput").ap()
    wvT = nc.dram_tensor("wvT", [H, JC], F32R, kind="ExternalInput").ap()
    woJI = nc.dram_tensor("woJI", [JC, H], F32R, kind="ExternalInput").ap()
    cosT = nc.dram_tensor("cosT", [128, S], F32, kind="ExternalInput").ap()
    sinTs = nc.dram_tensor("sinTs", [128, S], F32, kind="ExternalInput").ap()
    out = nc.dram_tensor("out", [T, H], F32, kind="ExternalOutput").ap()

    with tile.TileContext(nc) as tc:
        with (
            tc.tile_pool(name="wts", bufs=1) as wts,
            tc.tile_pool(name="tabs", bufs=1) as tabs,
            tc.tile_pool(name="hst", bufs=9) as hst,
            tc.tile_pool(name="qkv", bufs=1) as qkvp,
            tc.tile_pool(name="ps", bufs=3, space="PSUM") as ps,
            tc.tile_pool(name="cxp", bufs=1, space="PSUM") as cxp,
            tc.tile_pool(name="rope", bufs=2) as ropep,
            tc.tile_pool(name="vaug", bufs=1) as vaugp,
            tc.tile_pool(name="expt", bufs=3) as exptp,
            tc.tile_pool(name="ctx", bufs=1) as ctxp,
            tc.tile_pool(name="nrm", bufs=3) as nrmp,
            tc.tile_pool(name="outs", bufs=3) as outsp,
        ):
            # ---- persistent weights / tables ----
            wq_sb = wts.tile([128, 8, JC], F32R, tag="wq")
            nc.sync.dma_start(
                out=wq_sb[:], in_=wqT[:, :].rearrange("(k p) j -> p k j", p=128)
            )
            wk_sb = wts.tile([128, 8, JC], F32R, tag="wk")
            nc.sync.dma_start(
                out=wk_sb[:], in_=wkT[:, :].rearrange("(k p) j -> p k j", p=128)
            )
            wv_sb = wts.tile([128, 8, JC], F32R, tag="wv")
            nc.sync.dma_start(
                out=wv_sb[:], in_=wvT[:, :].rearrange("(k p) j -> p k j", p=128)
            )
            wJ = wts.tile([128, H], F32R, tag="wj")
            nc.sync.dma_start(out=wJ[:], in_=woJI[:, :])
            cos_sb = tabs.tile([128, S], F32, tag="cos")
            nc.sync.dma_start(out=cos_sb[:], in_=cosT[:, :])
            sin_sb = tabs.tile([128, S], F32, tag="sin")
            nc.sync.dma_start(out=sin_sb[:], in_=sinTs[:, :])
            ident = tabs.tile([128, 128], F32, tag="ident")
            make_identity(nc, ident[:])
            onesc = tabs.tile([128, NKT], F32, tag="ones")
            nc.vector.memset(onesc[:], 1.0)
            ones64 = tabs.tile([1, 64], F32R, tag="ones64")
            nc.vector.tensor_copy(ones64[:], onesc[0:1, 0:1].to_broadcast([1, 64]))

            for b in range(B):
                # ======== QKV projections (+RoPE), feature-major ========
                qT = qkvp.tile([128, S], F16, tag="qT")
                kT = qkvp.tile([128, S], F16, tag="kT")
                vT = qkvp.tile([128, S], F32, tag="vT")

                for wv_i in range(S // WAVE):  # 2 waves of 1024 tokens
                    t0 = b * S + wv_i * WAVE
                    sl = slice(wv_i * WAVE, (wv_i + 1) * WAVE)
                    chunks = []
                    for k in range(8):
                        c = hst.tile([128, WAVE], F32R, tag="hst")
                        nc.sync.dma_start(
                            out=c[:], in_=hsT[128 * k : 128 * (k + 1), t0 : t0 + WAVE]
                        )
                        chunks.append(c)
                    pq = ps.tile([128, WAVE], F32, tag="ps")
                    pk = ps.tile([128, WAVE], F32, tag="ps")
                    pv = ps.tile([128, WAVE], F32, tag="ps")
                    for k in range(8):
                        st, sp = k == 0, k == 7
                        for w_sb, p in ((wq_sb, pq), (wk_sb, pk), (wv_sb, pv)):
                            for hf in range(2):
                                fsl = slice(hf * 512, (hf + 1) * 512)
                                nc.tensor.matmul(
                                    p[:, fsl],
                                    w_sb[:, k, :],
                                    chunks[k][:, fsl],
                                    start=st,
                                    stop=sp,
                                )
                    # v: plain copy to SBUF (f32)
                    nc.vector.tensor_copy(vT[:, sl], pv[:])
                    # rope(q), rope(k)
                    for psrc, dstT in ((pq, qT), (pk, kT)):
                        raw = ropep.tile([128, WAVE], F32, tag="raw")
                        nc.vector.tensor_copy(raw[:], psrc[:])
                        rot = ropep.tile([128, WAVE], F32, tag="rot")
                        for h0 in (0, 64):
                            nc.sync.dma_start(
                                out=rot[h0 : h0 + 32, :], in_=raw[h0 + 32 : h0 + 64, :]
                            )
                            nc.sync.dma_start(
                                out=rot[h0 + 32 : h0 + 64, :], in_=raw[h0 : h0 + 32, :]
                            )
                        t1 = ropep.tile([128, WAVE], F32, tag="t1")
                        nc.vector.tensor_mul(t1[:], raw[:], cos_sb[:, sl])
                        t2 = ropep.tile([128, WAVE], F32, tag="t2")
                        nc.vector.tensor_mul(t2[:], rot[:], sin_sb[:, sl])
                        nc.vector.tensor_add(dstT[:, sl], t1[:], t2[:])

                # ======== v transpose -> per-head augmented V ========
                vA = vaugp.tile([128, NKT, 65], F16, tag="vA")
                vB = vaugp.tile([128, NKT, 65], F16, tag="vB")
                nc.vector.tensor_copy(vA[:, :, 64], onesc[:])
                nc.vector.tensor_copy(vB[:, :, 64], onesc[:])
                for tkb in range(NKT):
                    pt = ps.tile([128, WAVE], F32, tag="ps")
                    nc.tensor.transpose(
                        pt[:, 0:128], vT[:, 128 * tkb : 128 * (tkb + 1)], ident[:]
                    )
                    nc.vector.tensor_copy(vA[:, tkb, 0:64], pt[:, 0:64])
                    nc.vector.tensor_copy(vB[:, tkb, 0:64], pt[:, 64:128])

                # ======== attention: scoresT -> exp -> ctxT ========
                ctxS = ctxp.tile([128, S], F32R, tag="cts")
                ctxB = ctxp.tile([64, S], F32R, tag="ctb")
                ctxA = ctxS
                for tqb in range(S // TQB):
                    qsl = slice(tqb * TQB, (tqb + 1) * TQB)
                    for hh, (h0, vaug, ctxT) in enumerate(
                        ((0, vA, ctxA), (64, vB, ctxB))
                    ):
                        cx = cxp.tile([65, TQB], F32, tag="cx" + "ab"[hh])
                        for p in range(NKT // 2):
                            sc = ps.tile([128, 2 * TQB], F32, tag="ps")
                            for t in range(2):
                                tkb = 2 * p + t
                                nc.tensor.matmul(
                                    sc[:, t * TQB : (t + 1) * TQB],
                                    kT[h0 : h0 + 64, 128 * tkb : 128 * (tkb + 1)],
                                    qT[h0 : h0 + 64, qsl],
                                    start=True,
                                    stop=True,
                                )
                            et = exptp.tile([128, 2 * TQB], F16, tag="et")
                            nc.scalar.activation(et[:], sc[:], EXP, scale=0.125)
                            for t in range(2):
                                tkb = 2 * p + t
                                nc.tensor.matmul(
                                    cx[:],
                                    vaug[:, tkb, :],
                                    et[:, t * TQB : (t + 1) * TQB],
                                    start=(tkb == 0),
                                    stop=(tkb == NKT - 1),
                                )
                        # ctxT[:, qsl] = cx[0:64] / Z   (Z = row 64)
                        rz = nrmp.tile([1, TQB], F32R, tag="rz")
                        with nc.allow_low_precision(reason="f32r softmax norm"):
                            nc.vector.reciprocal(rz[:], cx[64:65, :])
                        pz = ps.tile([128, WAVE], F32, tag="ps")
                        nc.tensor.matmul(
                            pz[0:64, 0:TQB], ones64[:], rz[:], start=True, stop=True
                        )
                        zrep = nrmp.tile([64, TQB], F32, tag="zrep")
                        nc.vector.tensor_copy(zrep[:], pz[0:64, 0:TQB])
                        dst = ctxT[0:64, qsl] if ctxT is ctxS else ctxT[:, qsl]
                        nc.vector.tensor_mul(dst, cx[0:64, :], zrep[:])

                nc.sync.dma_start(out=ctxS[64:128, :], in_=ctxB[:, :])

                # ======== output projection (natural-layout out) ========
                for tq8 in range(S // 128):
                    po = ps.tile([128, WAVE], F32, tag="ps")
                    csl = slice(128 * tq8, 128 * (tq8 + 1))
                    for ich in range(2):
                        isl = slice(ich * 512, (ich + 1) * 512)
                        nc.tensor.matmul(
                            po[:, isl], ctxS[:, csl], wJ[:, isl], start=True, stop=True
                        )
                    ot = outsp.tile([128, H], F32, tag="ot")
                    if tq8 % 2 == 0:
                        nc.vector.tensor_copy(ot[:], po[:])
                    else:
                        nc.scalar.copy(ot[:], po[:])
                    nc.sync.dma_start(
                        out=out[b * S + 128 * tq8 : b * S + 128 * (tq8 + 1), :],
                        in_=ot[:],
                    )

    nc.compile()
    return nc


def _rope_tables():
    inv_freq = 1.0 / (BASE ** (np.arange(0, HD, 2, dtype=np.float64) / HD))
    t = np.arange(S, dtype=np.float64)
    freqs = np.outer(t, inv_freq)  # [S, 32]
    emb = np.concatenate([freqs, freqs], -1)  # [S, 64]
    cos = np.cos(emb).T.astype(np.float32)  # [64, S]
    sin = np.sin(emb).T.astype(np.float32)
    sin_signed = sin.copy()
    sin_signed[0:32] = -sin_signed[0:32]
    cosT = np.ascontiguousarray(np.tile(cos, (2, 1)))  # [128, S]
    sinTs = np.ascontiguousarray(np.tile(sin_signed, (2, 1)))
    return cosT, sinTs


def kernel(hidden_states, Wq, Wk, Wv, Wo):
    hidden_states = np.asarray(hidden_states, np.float32)
    Wq, Wk, Wv, Wo = (np.asarray(w, np.float32) for w in (Wq, Wk, Wv, Wo))

    if _nc_cache[0] is None:
        _nc_cache[0] = _build()
    nc = _nc_cache[0]

    hsT = np.ascontiguousarray(hidden_states.reshape(T, H).T)  # [H, T]
    cosT, sinTs = _rope_tables()
    in_maps = []
    for c in range(NCORES):
        sl = slice(JC * c, JC * (c + 1))
        in_maps.append(
            {
                "hsT": hsT,
                "wqT": np.ascontiguousarray(Wq[sl, :].T),
                "wkT": np.ascontiguousarray(Wk[sl, :].T),
                "wvT": np.ascontiguousarray(Wv[sl, :].T),
                "woJI": np.ascontiguousarray(Wo[:, sl].T),
                "cosT": cosT,
                "sinTs": sinTs,
            }
        )

    from concourse.bass_utils import run_bass_kernel_spmd

    res = run_bass_kernel_spmd(nc, in_maps, core_ids=list(range(NCORES)))
    acc = np.zeros((T, H), np.float64)
    for c in range(NCORES):
        acc += res.results[c]["out"]
    return acc.astype(np.float32).reshape(B, S, H)
